# revision 1
# baseline (speedup 1.0000x reference)
"""Trainium2 Bass kernel for nn_CpSae_44014824849572.

Computes the CP-SAE loss. The reference materializes a [1024, 64, 32, 32]
CP-reconstruction `volume` and diffs it against `features`. We instead use

  sum((flat - volume)^2) = sum(flat^2) - 2*sum(flat*volume) + sum(volume^2)

with  sum(flat*volume)[b] = sum_z a[b,z] * T[b,z],
      T[b,z]   = sum_feat flat[b,feat] * KRP[g_b][z,feat]
      KRP[g]   = softplus(freq)⊗softplus(roi1)⊗softplus(roi2)  (rank-1 rows)
      sum(volume^2)[b] = a_b^T M_{g_b} a_b,
      M_g = (Ff Ff^T) ∘ (R1 R1^T) ∘ (R2 R2^T)   (32x32 per group, tiny)

so the only heavy device work is two big contractions over the feature dim:
  zcat[b, 0:64] = flat[b] @ [W1 | W2]          (encoder, 8.6 GFLOP)
  T[b, z]       = flat[b] @ KRP[g_b].T         (4.3 GFLOP)

Distribution: feature-dim sharded across 8 cores (8192 features each, all
1024 samples -> moving free dim of 512 per matmul). Samples are sorted by
group on the host so each group's T-matmul sees a contiguous column block
with one shared stationary operand; groups are packed 4-at-a-time into the
128 PE columns. Encoder matmuls for even/odd k-chunks run concurrently on
disjoint PE column halves via tile_position. Data is fp8e4m3 (weights
pre-scaled) with fp32 PSUM accumulation; partial zcat/T are summed on host.
"""
import json

import numpy as np
import ml_dtypes

import concourse.bass as bass
import concourse.mybir as mybir
import concourse.tile as tile
from concourse.bass_utils import run_bass_kernel_spmd

N_CORES = 8
BATCH = 1024
N_FREQS = 64
N_ROIS = 32
Z = 32
N_GROUPS = 16
N_CLASSES = 4
N_FEAT = N_FREQS * N_ROIS * N_ROIS          # 65536
FEAT_PER_CORE = N_FEAT // N_CORES           # 8192
KCHUNKS = FEAT_PER_CORE // 128              # 64
CHUNKS_PER_DMA = 2
KRP_SLICES = 8
EPSILON = 1e-06
REG_STRENGTH = 1.0
KL_FACTOR = 1.0

F32 = mybir.dt.float32
DATA_MODE = "fp8"                           # "fp8" | "bf16"
if DATA_MODE == "fp8":
    DT = mybir.dt.float8e4
    NPDT = ml_dtypes.float8_e4m3
    W_SCALE = 4096.0
else:
    DT = mybir.dt.bfloat16
    NPDT = ml_dtypes.bfloat16
    W_SCALE = 1.0
NPBYTES = np.dtype(NPDT).itemsize
_U = np.uint8 if NPBYTES == 1 else np.uint16

_waitfix_counter = [0]


def _split_waits_in_bir(bir: dict) -> int:
    """This container's walrus accepts only ONE sync wait per instruction;
    Tile emits several. Hoist all-but-one wait onto EventSemaphore
    instructions inserted just before, on the same engine."""
    nsplit = 0
    for fn in bir.get("functions", []):
        for blk in fn.get("blocks", []):
            out = []
            for insn in blk.get("instructions", []):
                si = insn.get("sync_info") or {}
                ow = si.get("on_wait") or []
                if len(ow) > 1:
                    for w in ow[:-1]:
                        _waitfix_counter[0] += 1
                        out.append({
                            "debug": insn.get("debug", 0),
                            "engine": insn["engine"],
                            "ins": [],
                            "name": f"{insn['name']}-wsplit{_waitfix_counter[0]}",
                            "opcode": "EventSemaphore",
                            "outs": [],
                            "sync_info": {"on_update": [], "on_wait": [w]},
                        })
                        nsplit += 1
                    si["on_wait"] = [ow[-1]]
                out.append(insn)
            blk["instructions"] = out
    return nsplit


def _install_waitfix():
    import concourse.bass2jax as bass2jax
    import concourse.bass_utils as bass_utils

    if getattr(bass2jax, "_waitfix_installed", False):
        return
    orig = bass_utils.compile_bir_kernel

    def patched(bir_json, tmpdir, neff_name="file.neff"):
        bir = json.loads(bir_json.decode() if isinstance(bir_json, bytes) else bir_json)
        _split_waits_in_bir(bir)
        return orig(json.dumps(bir).encode(), tmpdir, neff_name)

    bass2jax.compile_bir_kernel = patched
    bass_utils.compile_bir_kernel = patched
    bass2jax._waitfix_installed = True


def _softplus(x):
    return np.logaddexp(0.0, x.astype(np.float64)).astype(np.float32)


def _quartet_blocks(groups_sorted):
    """[(q, c0, c1)] contiguous column blocks (<=512 wide) per group-quartet
    q (groups 4q..4q+3)."""
    gs = np.asarray(groups_sorted)
    blocks = []
    for q in range(N_GROUPS // 4):
        c0 = int(np.searchsorted(gs, 4 * q))
        c1 = int(np.searchsorted(gs, 4 * q + 4))
        while c0 < c1:
            ce = min(c0 + 512, c1)
            blocks.append((q, c0, ce))
            c0 = ce
    return blocks


def build_device_program(blocks, parts="all"):
    """One SPMD program (shared by all 8 cores). Per-core inputs:
      flatt [KCHUNKS, 128, BATCH]  — transposed feature slice (group-sorted)
      w     [128, KCHUNKS, 64]     — [W1|W2]*W_SCALE slice, partition-major
      krpt  [128, KCHUNKS, 16, Z]  — KRP slice, partition-major
    Outputs (partial sums over this core's features):
      zcat [128, BATCH] f32 — rows 0:64 even-k half, 64:128 odd-k half
      tt   [128, BATCH] f32 — row (g%4)*32+z holds T[z] for that column's group
    """
    nc = bass.Bass()
    flatt = nc.dram_tensor("flatt", [KCHUNKS, 128, BATCH], DT, kind="ExternalInput")
    w = nc.dram_tensor("w", [128, KCHUNKS, 64], DT, kind="ExternalInput")
    krpt = nc.dram_tensor("krpt", [128, KCHUNKS, N_GROUPS, Z], DT, kind="ExternalInput")
    zcat_out = nc.dram_tensor("zcat", [128, BATCH], F32, kind="ExternalOutput")
    tt_out = nc.dram_tensor("tt", [128, BATCH], F32, kind="ExternalOutput")

    kc_per_slice = KCHUNKS // KRP_SLICES

    with tile.TileContext(nc) as tc:
        with (
            tc.tile_pool(name="fpool", bufs=6) as fpool,
            tc.tile_pool(name="const", bufs=1) as const,
            tc.tile_pool(name="opool", bufs=1) as opool,
            tc.tile_pool(name="psum", bufs=1, space="PSUM") as psum,
        ):
            wt = const.tile([128, KCHUNKS, 64], DT, tag="w")
            if parts != "pe":
                nc.sync.dma_start(out=wt, in_=w[:, :, :])
            krp_tiles = []
            for j in range(KRP_SLICES):
                kt = const.tile([128, kc_per_slice, N_GROUPS, Z], DT, tag=f"krp{j}")
                if parts != "pe":
                    nc.sync.dma_start(
                        out=kt,
                        in_=krpt[:, j * kc_per_slice:(j + 1) * kc_per_slice, :, :],
                    )
                krp_tiles.append(kt)

            zcat_ps = t_ps = None
            if parts != "dma":
                zcat_ps = psum.tile([128, BATCH], F32, tag="zcat")
                t_ps = psum.tile([128, BATCH], F32, tag="t")

            for k0 in range(0, KCHUNKS, CHUNKS_PER_DMA):
                nch = min(CHUNKS_PER_DMA, KCHUNKS - k0)
                ft = fpool.tile([128, nch, BATCH], DT, tag="flat")
                if parts != "pe":
                    nc.sync.dma_start(
                        out=ft, in_=flatt[k0:k0 + nch, :, :].rearrange("c p n -> p c n")
                    )
                for kk in range(nch):
                    k = k0 + kk
                    if parts == "dma":
                        continue
                    start = k < 2
                    stop = k >= KCHUNKS - 2
                    par = k % 2
                    # encoder: even/odd k-chunks on disjoint PE column halves
                    for half in range(2):
                        nc.tensor.matmul(
                            zcat_ps[par * 64:(par + 1) * 64,
                                    half * 512:(half + 1) * 512],
                            wt[:, k, :],
                            ft[:, kk, half * 512:(half + 1) * 512],
                            start=start,
                            stop=stop,
                            tile_position=(0, par * 64),
                        )
                    # T: 4 groups packed into the 128 PE columns per matmul
                    kt = krp_tiles[k // kc_per_slice]
                    kloc = k % kc_per_slice
                    for (q, c0, c1) in blocks:
                        nc.tensor.matmul(
                            t_ps[:, c0:c1],
                            kt[:, kloc, 4 * q:4 * (q + 1), :],
                            ft[:, kk, c0:c1],
                            start=(k == 0),
                            stop=(k == KCHUNKS - 1),
                        )

            if parts != "dma":
                zc_sb = opool.tile([128, BATCH], F32, tag="zc")
                nc.vector.tensor_copy(zc_sb, zcat_ps)
                nc.sync.dma_start(out=zcat_out[:, :], in_=zc_sb)
                tt_sb = opool.tile([128, BATCH], F32, tag="tt")
                nc.scalar.copy(tt_sb, t_ps)
                nc.sync.dma_start(out=tt_out[:, :], in_=tt_sb)
    return nc


def _prepare(inputs):
    features = np.asarray(inputs["features"], dtype=np.float32)
    labels = np.asarray(inputs["labels"]).astype(np.int64)
    groups = np.asarray(inputs["groups"]).astype(np.int64)
    weights = np.asarray(inputs["weights"], dtype=np.float32)
    noise = np.asarray(inputs["noise"], dtype=np.float32)
    group_embed = np.asarray(inputs["group_embed"], dtype=np.float32)
    W1 = np.asarray(inputs["W1"], dtype=np.float32)
    b1 = np.asarray(inputs["b1"], dtype=np.float32)
    W2 = np.asarray(inputs["W2"], dtype=np.float32)
    b2 = np.asarray(inputs["b2"], dtype=np.float32)
    freq_factors = np.asarray(inputs["freq_factors"], dtype=np.float32)
    roi_1_factors = np.asarray(inputs["roi_1_factors"], dtype=np.float32)
    roi_2_factors = np.asarray(inputs["roi_2_factors"], dtype=np.float32)
    lin_W = np.asarray(inputs["lin_W"], dtype=np.float32)
    lin_b = np.asarray(inputs["lin_b"], dtype=np.float32)
    logit_bias = np.asarray(inputs["logit_bias"], dtype=np.float32)

    b = features.shape[0]
    flat = features.reshape(b, -1)

    perm = np.argsort(groups, kind="stable")
    groups_sorted = groups[perm]
    blocks = _quartet_blocks(groups_sorted)

    sq = np.einsum("bi,bi->b", flat, flat, optimize=True)

    flat_q = flat[perm].astype(NPDT)
    flatT = flat_q.view(_U).T.copy().view(NPDT)            # [N_FEAT, BATCH]

    W = (np.concatenate([W1[:N_FEAT], W2[:N_FEAT]], axis=1) * W_SCALE).astype(NPDT)

    Ff = _softplus(freq_factors)
    R1 = _softplus(roi_1_factors)
    R2 = _softplus(roi_2_factors)

    krp = np.empty((N_GROUPS, Z, N_FEAT), dtype=NPDT)
    for g in range(N_GROUPS):
        r12 = np.einsum("zr,zs->zrs", R1[g], R2[g]).reshape(Z, N_ROIS * N_ROIS)
        krp[g] = np.einsum("zf,zx->zfx", Ff[g], r12).reshape(Z, N_FEAT).astype(NPDT)
    krpt = krp.view(_U).transpose(2, 0, 1).reshape(N_CORES, KCHUNKS, 128, N_GROUPS, Z)
    krpt = krpt.transpose(0, 2, 1, 3, 4).copy().view(NPDT)

    w_dev = W.view(_U).reshape(N_CORES, KCHUNKS, 128, 64)
    w_dev = w_dev.transpose(0, 2, 1, 3).copy().view(NPDT)

    in_maps = []
    for c in range(N_CORES):
        in_maps.append({
            "flatt": np.ascontiguousarray(
                flatT[c * FEAT_PER_CORE:(c + 1) * FEAT_PER_CORE].view(_U)
            ).reshape(KCHUNKS, 128, BATCH).view(NPDT),
            "w": w_dev[c],
            "krpt": krpt[c],
        })

    host = dict(
        labels=labels, groups=groups, weights=weights, noise=noise,
        group_embed=group_embed, W1=W1, b1=b1, W2=W2, b2=b2,
        lin_W=lin_W, lin_b=lin_b, logit_bias=logit_bias,
        Ff=Ff, R1=R1, R2=R2, sq=sq, perm=perm, b=b,
        groups_sorted=groups_sorted,
    )
    return in_maps, blocks, host


def _finish(zcatT, ttT, host):
    b = host["b"]
    perm = host["perm"]
    inv = np.empty_like(perm)
    inv[perm] = np.arange(b)

    zcat = ((zcatT[:64] + zcatT[64:]) / W_SCALE).T[inv]    # [b, 64]
    gs4 = (host["groups_sorted"] % 4).astype(np.int64)     # row quartet per sorted col
    cols = np.arange(b)
    T_sorted = np.empty((b, Z), np.float32)
    for zi in range(Z):
        T_sorted[:, zi] = ttT[gs4 * Z + zi, cols]
    T = T_sorted[inv]

    groups = host["groups"]
    ge = host["group_embed"][groups]
    z_mu = zcat[:, :Z] + host["b1"] + ge @ host["W1"][N_FEAT:]
    z_log_std = zcat[:, Z:] + host["b2"] + ge @ host["W2"][N_FEAT:]
    sigma = EPSILON + np.exp(z_log_std)
    kld = np.sum(-np.log(sigma) + 0.5 * (sigma * sigma + z_mu * z_mu - 1.0), axis=1)
    zs = z_mu + sigma * host["noise"]
    zs = zs @ host["lin_W"] + host["lin_b"]
    a = _softplus(zs)

    Ff, R1, R2 = host["Ff"], host["R1"], host["R2"]
    M = (np.einsum("gzf,gyf->gzy", Ff, Ff)
         * np.einsum("gzr,gyr->gzy", R1, R1)
         * np.einsum("gzs,gys->gzy", R2, R2))
    vol2 = np.einsum("bz,bzy,by->b", a, M[groups], a)
    fdotv = np.sum(a * T, axis=1)
    rec = REG_STRENGTH * (host["sq"] - 2.0 * fdotv + vol2) / N_FEAT

    logits = np.concatenate([zs[:, :N_CLASSES - 1], np.ones((b, 1), np.float32)],
                            axis=1) + host["logit_bias"]
    m = logits.max(axis=1, keepdims=True)
    lse = m[:, 0] + np.log(np.exp(logits - m).sum(axis=1))
    log_probs = logits[np.arange(b), host["labels"]] - lse

    freq_loss = np.var(Ff, axis=0, ddof=1).mean(axis=1).sum()
    roi_loss = (np.var(R1, axis=0, ddof=1) + np.var(R2, axis=0, ddof=1)).mean(axis=1).sum()

    loss = np.mean(rec - host["weights"] * log_probs + KL_FACTOR * kld) \
        + freq_loss + roi_loss
    return np.float32(loss)


def kernel(**inputs) -> np.ndarray:
    _install_waitfix()
    in_maps, blocks, host = _prepare(inputs)
    nc = build_device_program(blocks)
    r = run_bass_kernel_spmd(nc, in_maps, core_ids=list(range(N_CORES)))
    zcatT = np.zeros((128, BATCH), np.float32)
    ttT = np.zeros((128, BATCH), np.float32)
    for c in range(N_CORES):
        zcatT += r.results[c]["zcat"]
        ttT += r.results[c]["tt"]
    return _finish(zcatT, ttT, host)



# revision 6
# speedup vs baseline: 1.8061x; 1.8061x over previous
"""Trainium2 Bass kernel for nn_CpSae_44014824849572.

Computes the CP-SAE loss. The reference materializes a [1024, 64, 32, 32]
CP-reconstruction `volume` and diffs it against `features`. We instead use

  sum((flat - volume)^2) = sum(flat^2) - 2*sum(flat*volume) + sum(volume^2)

with  sum(flat*volume)[b] = sum_z a[b,z] * T[b,z],
      T[b,z]   = sum_feat flat[b,feat] * KRP[g_b][z,feat]
      KRP[g]   = softplus(freq)⊗softplus(roi1)⊗softplus(roi2)  (rank-1 rows)
      sum(volume^2)[b] = a_b^T M_{g_b} a_b,
      M_g = (Ff Ff^T) ∘ (R1 R1^T) ∘ (R2 R2^T)   (32x32 per group, tiny)

so the only heavy device work is two big contractions over the feature dim:
  zcat[b, 0:64] = flat[b] @ [W1 | W2]          (encoder, 8.6 GFLOP)
  T[b, z]       = flat[b] @ KRP[g_b].T         (4.3 GFLOP)

Distribution: feature-dim sharded across 8 cores (8192 features each, all
1024 samples -> moving free dim of 512 per matmul). Samples are sorted by
group on the host so each group's T-matmul sees a contiguous column block
with one shared stationary operand; groups are packed 4-at-a-time into the
128 PE columns. Encoder matmuls for even/odd k-chunks run concurrently on
disjoint PE column halves via tile_position. Data is fp8e4m3 (weights
pre-scaled) with fp32 PSUM accumulation; partial zcat/T are summed on host.
"""
import json

import numpy as np
import ml_dtypes

import concourse.bass as bass
import concourse.mybir as mybir
import concourse.tile as tile
from concourse.bass_utils import run_bass_kernel_spmd

N_CORES = 8
BATCH = 1024
N_FREQS = 64
N_ROIS = 32
Z = 32
N_GROUPS = 16
N_CLASSES = 4
N_FEAT = N_FREQS * N_ROIS * N_ROIS          # 65536
FEAT_PER_CORE = N_FEAT // N_CORES           # 8192
KCHUNKS = FEAT_PER_CORE // 128              # 64
CHUNKS_PER_DMA = 2
KRP_SLICES = 8
EPSILON = 1e-06
REG_STRENGTH = 1.0
KL_FACTOR = 1.0

F32 = mybir.dt.float32
DATA_MODE = "fp8"                           # "fp8" | "bf16"
if DATA_MODE == "fp8":
    DT = mybir.dt.float8e4
    NPDT = ml_dtypes.float8_e4m3
    W_SCALE = 4096.0
else:
    DT = mybir.dt.bfloat16
    NPDT = ml_dtypes.bfloat16
    W_SCALE = 1.0
NPBYTES = np.dtype(NPDT).itemsize
_U = np.uint8 if NPBYTES == 1 else np.uint16

_waitfix_counter = [0]


def _split_waits_in_bir(bir: dict) -> int:
    """This container's walrus accepts only ONE sync wait per instruction;
    Tile emits several. Hoist all-but-one wait onto EventSemaphore
    instructions inserted just before, on the same engine."""
    nsplit = 0
    for fn in bir.get("functions", []):
        for blk in fn.get("blocks", []):
            out = []
            for insn in blk.get("instructions", []):
                si = insn.get("sync_info") or {}
                ow = si.get("on_wait") or []
                if len(ow) > 1:
                    for w in ow[:-1]:
                        _waitfix_counter[0] += 1
                        out.append({
                            "debug": insn.get("debug", 0),
                            "engine": insn["engine"],
                            "ins": [],
                            "name": f"{insn['name']}-wsplit{_waitfix_counter[0]}",
                            "opcode": "EventSemaphore",
                            "outs": [],
                            "sync_info": {"on_update": [], "on_wait": [w]},
                        })
                        nsplit += 1
                    si["on_wait"] = [ow[-1]]
                out.append(insn)
            blk["instructions"] = out
    return nsplit


def _install_waitfix():
    import concourse.bass2jax as bass2jax
    import concourse.bass_utils as bass_utils

    if getattr(bass2jax, "_waitfix_installed", False):
        return
    orig = bass_utils.compile_bir_kernel

    def patched(bir_json, tmpdir, neff_name="file.neff"):
        bir = json.loads(bir_json.decode() if isinstance(bir_json, bytes) else bir_json)
        _split_waits_in_bir(bir)
        return orig(json.dumps(bir).encode(), tmpdir, neff_name)

    bass2jax.compile_bir_kernel = patched
    bass_utils.compile_bir_kernel = patched
    bass2jax._waitfix_installed = True


def _softplus(x):
    return np.logaddexp(0.0, x.astype(np.float64)).astype(np.float32)


def _quartet_blocks(groups_sorted):
    """[(q, c0, c1)] contiguous column blocks (<=512 wide) per group-quartet
    q (groups 4q..4q+3)."""
    gs = np.asarray(groups_sorted)
    blocks = []
    for q in range(N_GROUPS // 4):
        c0 = int(np.searchsorted(gs, 4 * q))
        c1 = int(np.searchsorted(gs, 4 * q + 4))
        while c0 < c1:
            ce = min(c0 + 512, c1)
            blocks.append((q, c0, ce))
            c0 = ce
    return blocks


def build_device_program(blocks, parts="all"):
    """One SPMD program (shared by all 8 cores). Per-core inputs:
      flatt [KCHUNKS, 128, BATCH]  — transposed feature slice (group-sorted)
      w     [128, KCHUNKS, 64]     — [W1|W2]*W_SCALE slice, partition-major
      krpt  [128, KCHUNKS, 16, Z]  — KRP slice, partition-major
    Outputs (partial sums over this core's features):
      zcat [128, BATCH] f32 — rows 0:64 even-k half, 64:128 odd-k half
      tt   [128, BATCH] f32 — row (g%4)*32+z holds T[z] for that column's group
    """
    nc = bass.Bass()
    flatt = nc.dram_tensor("flatt", [KCHUNKS, 128, BATCH], DT, kind="ExternalInput")
    w = nc.dram_tensor("w", [128, KCHUNKS, 64], DT, kind="ExternalInput")
    krpt = nc.dram_tensor("krpt", [128, KCHUNKS, N_GROUPS, Z], DT, kind="ExternalInput")
    zcat_out = nc.dram_tensor("zcat", [64, BATCH], F32, kind="ExternalOutput")
    tt_out = nc.dram_tensor("tt", [128, BATCH], F32, kind="ExternalOutput")

    kc_per_slice = KCHUNKS // KRP_SLICES

    with tile.TileContext(nc) as tc:
        with (
            tc.tile_pool(name="fpool", bufs=6) as fpool,
            tc.tile_pool(name="const", bufs=1) as const,
            tc.tile_pool(name="opool", bufs=1) as opool,
            tc.tile_pool(name="psum", bufs=1, space="PSUM") as psum,
        ):
            wt = const.tile([128, KCHUNKS, 64], DT, tag="w")
            if parts != "pe":
                nc.sync.dma_start(out=wt, in_=w[:, :, :])
            krp_tiles = []
            for j in range(KRP_SLICES):
                kt = const.tile([128, kc_per_slice, N_GROUPS, Z], DT, tag=f"krp{j}")
                if parts != "pe":
                    nc.sync.dma_start(
                        out=kt,
                        in_=krpt[:, j * kc_per_slice:(j + 1) * kc_per_slice, :, :],
                    )
                krp_tiles.append(kt)

            zcat_ps = t_ps = None
            if parts != "dma":
                zcat_ps = psum.tile([64, BATCH], F32, tag="zcat")
                t_ps = psum.tile([128, BATCH], F32, tag="t")

            DR = mybir.MatmulPerfMode.DoubleRow
            for k0 in range(0, KCHUNKS, CHUNKS_PER_DMA):
                ft = fpool.tile([128, CHUNKS_PER_DMA, BATCH], DT, tag="flat")
                if parts != "pe":
                    nc.sync.dma_start(
                        out=ft,
                        in_=flatt[k0:k0 + CHUNKS_PER_DMA, :, :].rearrange("c p n -> p c n"),
                    )
                if parts == "dma":
                    continue
                start = k0 == 0
                stop = k0 >= KCHUNKS - 2
                # encoder: one DoubleRow matmul consumes the k-chunk pair
                for half in range(2):
                    nc.tensor.matmul(
                        zcat_ps[:, half * 512:(half + 1) * 512],
                        wt[:, k0:k0 + 2, :],
                        ft[:, :, half * 512:(half + 1) * 512],
                        start=start,
                        stop=stop,
                        perf_mode=DR,
                    )
                # T: 4 groups packed into the 128 PE columns per matmul
                kt = krp_tiles[k0 // kc_per_slice]
                kloc = k0 % kc_per_slice
                for (q, c0, c1) in blocks:
                    nc.tensor.matmul(
                        t_ps[:, c0:c1],
                        kt[:, kloc:kloc + 2, 4 * q:4 * (q + 1), :],
                        ft[:, :, c0:c1],
                        start=start,
                        stop=stop,
                        perf_mode=DR,
                    )

            if parts != "dma":
                zc_sb = opool.tile([64, BATCH], F32, tag="zc")
                nc.vector.tensor_copy(zc_sb, zcat_ps)
                nc.sync.dma_start(out=zcat_out[:, :], in_=zc_sb)
                tt_sb = opool.tile([128, BATCH], F32, tag="tt")
                nc.scalar.copy(tt_sb, t_ps)
                nc.sync.dma_start(out=tt_out[:, :], in_=tt_sb)
    return nc


def _prepare(inputs):
    features = np.asarray(inputs["features"], dtype=np.float32)
    labels = np.asarray(inputs["labels"]).astype(np.int64)
    groups = np.asarray(inputs["groups"]).astype(np.int64)
    weights = np.asarray(inputs["weights"], dtype=np.float32)
    noise = np.asarray(inputs["noise"], dtype=np.float32)
    group_embed = np.asarray(inputs["group_embed"], dtype=np.float32)
    W1 = np.asarray(inputs["W1"], dtype=np.float32)
    b1 = np.asarray(inputs["b1"], dtype=np.float32)
    W2 = np.asarray(inputs["W2"], dtype=np.float32)
    b2 = np.asarray(inputs["b2"], dtype=np.float32)
    freq_factors = np.asarray(inputs["freq_factors"], dtype=np.float32)
    roi_1_factors = np.asarray(inputs["roi_1_factors"], dtype=np.float32)
    roi_2_factors = np.asarray(inputs["roi_2_factors"], dtype=np.float32)
    lin_W = np.asarray(inputs["lin_W"], dtype=np.float32)
    lin_b = np.asarray(inputs["lin_b"], dtype=np.float32)
    logit_bias = np.asarray(inputs["logit_bias"], dtype=np.float32)

    b = features.shape[0]
    flat = features.reshape(b, -1)

    perm = np.argsort(groups, kind="stable")
    groups_sorted = groups[perm]
    blocks = _quartet_blocks(groups_sorted)

    sq = np.einsum("bi,bi->b", flat, flat, optimize=True)

    flat_q = flat[perm].astype(NPDT)
    flatT = flat_q.view(_U).T.copy().view(NPDT)            # [N_FEAT, BATCH]

    W = (np.concatenate([W1[:N_FEAT], W2[:N_FEAT]], axis=1) * W_SCALE).astype(NPDT)

    Ff = _softplus(freq_factors)
    R1 = _softplus(roi_1_factors)
    R2 = _softplus(roi_2_factors)

    krp = np.empty((N_GROUPS, Z, N_FEAT), dtype=NPDT)
    for g in range(N_GROUPS):
        r12 = np.einsum("zr,zs->zrs", R1[g], R2[g]).reshape(Z, N_ROIS * N_ROIS)
        krp[g] = np.einsum("zf,zx->zfx", Ff[g], r12).reshape(Z, N_FEAT).astype(NPDT)
    krpt = krp.view(_U).transpose(2, 0, 1).reshape(N_CORES, KCHUNKS, 128, N_GROUPS, Z)
    krpt = krpt.transpose(0, 2, 1, 3, 4).copy().view(NPDT)

    w_dev = W.view(_U).reshape(N_CORES, KCHUNKS, 128, 64)
    w_dev = w_dev.transpose(0, 2, 1, 3).copy().view(NPDT)

    in_maps = []
    for c in range(N_CORES):
        in_maps.append({
            "flatt": np.ascontiguousarray(
                flatT[c * FEAT_PER_CORE:(c + 1) * FEAT_PER_CORE].view(_U)
            ).reshape(KCHUNKS, 128, BATCH).view(NPDT),
            "w": w_dev[c],
            "krpt": krpt[c],
        })

    host = dict(
        labels=labels, groups=groups, weights=weights, noise=noise,
        group_embed=group_embed, W1=W1, b1=b1, W2=W2, b2=b2,
        lin_W=lin_W, lin_b=lin_b, logit_bias=logit_bias,
        Ff=Ff, R1=R1, R2=R2, sq=sq, perm=perm, b=b,
        groups_sorted=groups_sorted,
    )
    return in_maps, blocks, host


def _finish(zcatT, ttT, host):
    b = host["b"]
    perm = host["perm"]
    inv = np.empty_like(perm)
    inv[perm] = np.arange(b)

    zcat = (zcatT / W_SCALE).T[inv]                        # [b, 64]
    gs4 = (host["groups_sorted"] % 4).astype(np.int64)     # row quartet per sorted col
    cols = np.arange(b)
    T_sorted = np.empty((b, Z), np.float32)
    for zi in range(Z):
        T_sorted[:, zi] = ttT[gs4 * Z + zi, cols]
    T = T_sorted[inv]

    groups = host["groups"]
    ge = host["group_embed"][groups]
    z_mu = zcat[:, :Z] + host["b1"] + ge @ host["W1"][N_FEAT:]
    z_log_std = zcat[:, Z:] + host["b2"] + ge @ host["W2"][N_FEAT:]
    sigma = EPSILON + np.exp(z_log_std)
    kld = np.sum(-np.log(sigma) + 0.5 * (sigma * sigma + z_mu * z_mu - 1.0), axis=1)
    zs = z_mu + sigma * host["noise"]
    zs = zs @ host["lin_W"] + host["lin_b"]
    a = _softplus(zs)

    Ff, R1, R2 = host["Ff"], host["R1"], host["R2"]
    M = (np.einsum("gzf,gyf->gzy", Ff, Ff)
         * np.einsum("gzr,gyr->gzy", R1, R1)
         * np.einsum("gzs,gys->gzy", R2, R2))
    vol2 = np.einsum("bz,bzy,by->b", a, M[groups], a)
    fdotv = np.sum(a * T, axis=1)
    rec = REG_STRENGTH * (host["sq"] - 2.0 * fdotv + vol2) / N_FEAT

    logits = np.concatenate([zs[:, :N_CLASSES - 1], np.ones((b, 1), np.float32)],
                            axis=1) + host["logit_bias"]
    m = logits.max(axis=1, keepdims=True)
    lse = m[:, 0] + np.log(np.exp(logits - m).sum(axis=1))
    log_probs = logits[np.arange(b), host["labels"]] - lse

    freq_loss = np.var(Ff, axis=0, ddof=1).mean(axis=1).sum()
    roi_loss = (np.var(R1, axis=0, ddof=1) + np.var(R2, axis=0, ddof=1)).mean(axis=1).sum()

    loss = np.mean(rec - host["weights"] * log_probs + KL_FACTOR * kld) \
        + freq_loss + roi_loss
    return np.float32(loss)


def kernel(**inputs) -> np.ndarray:
    _install_waitfix()
    in_maps, blocks, host = _prepare(inputs)
    nc = build_device_program(blocks)
    r = run_bass_kernel_spmd(nc, in_maps, core_ids=list(range(N_CORES)))
    zcatT = np.zeros((64, BATCH), np.float32)
    ttT = np.zeros((128, BATCH), np.float32)
    for c in range(N_CORES):
        zcatT += r.results[c]["zcat"]
        ttT += r.results[c]["tt"]
    return _finish(zcatT, ttT, host)



# revision 19
# speedup vs baseline: 2.2438x; 1.2423x over previous
"""Trainium2 Bass kernel for nn_CpSae_44014824849572.

Computes the CP-SAE loss. The reference materializes a [1024, 64, 32, 32]
CP-reconstruction `volume` and diffs it against `features`. We instead use

  sum((flat - volume)^2) = sum(flat^2) - 2*sum(flat*volume) + sum(volume^2)

with  sum(flat*volume)[b] = sum_z a[b,z] * T[b,z],
      T[b,z]   = sum_feat flat[b,feat] * KRP[g_b][z,feat]
      KRP[g]   = softplus(freq)⊗softplus(roi1)⊗softplus(roi2)  (rank-1 rows)
      sum(volume^2)[b] = a_b^T M_{g_b} a_b,
      M_g = (Ff Ff^T) ∘ (R1 R1^T) ∘ (R2 R2^T)   (32x32 per group, tiny)

so the only heavy device work is two big contractions over the feature dim:
  zcat[b, 0:64] = flat[b] @ [W1 | W2]          (encoder, 8.6 GFLOP)
  T[b, z]       = flat[b] @ KRP[g_b].T         (4.3 GFLOP)

Instead of materializing KRP (4MB/core in HBM), we exploit its rank-1
structure: within a 128-feature chunk (f1 fixed, 4 r1 values x 32 r2) the
KRP stationary is (R1⊗R2)[g] scaled by Ff[g,z,f1]. The device contracts
against the f1-independent RR = R1⊗R2 stationary (0.5MB, shared by all
cores) accumulating per-f1 partials U[f1][z,b], then folds the Ff weights
with stacked-identity matmuls:  T[z,b] = sum_f1 Ff[g_b,z,f1] * U[f1][z,b].

Distribution: feature-dim sharded across 8 cores (8192 features = 8 f1
values each, all 1024 samples). Samples are group-sorted on the host so
each group's U-matmul sees a contiguous column block. All heavy matmuls are
fp8 with MatmulPerfMode.DoubleRow (two 128-feature chunks contracted per
instruction). Partial zcat/T are summed on host.
"""
import json

import numpy as np
import ml_dtypes

import concourse.bass as bass
import concourse.mybir as mybir
import concourse.tile as tile
from concourse.bass_utils import run_bass_kernel_spmd

N_CORES = 8
BATCH = 1024
N_FREQS = 64
N_ROIS = 32
Z = 32
N_GROUPS = 16
N_CLASSES = 4
N_FEAT = N_FREQS * N_ROIS * N_ROIS          # 65536
FEAT_PER_CORE = N_FEAT // N_CORES           # 8192
KCHUNKS = FEAT_PER_CORE // 128              # 64
F1_PER_CORE = FEAT_PER_CORE // (N_ROIS * N_ROIS)  # 8
EPSILON = 1e-06
REG_STRENGTH = 1.0
KL_FACTOR = 1.0

F32 = mybir.dt.float32
BF16 = mybir.dt.bfloat16
DT = mybir.dt.float8e4
NPDT = ml_dtypes.float8_e4m3
W_SCALE = 4096.0

_waitfix_counter = [0]


def _split_waits_in_bir(bir: dict) -> int:
    """This container's walrus accepts only ONE sync wait per instruction;
    Tile emits several. Hoist all-but-one wait onto EventSemaphore
    instructions inserted just before, on the same engine."""
    nsplit = 0
    for fn in bir.get("functions", []):
        for blk in fn.get("blocks", []):
            out = []
            for insn in blk.get("instructions", []):
                si = insn.get("sync_info") or {}
                ow = si.get("on_wait") or []
                if len(ow) > 1:
                    for w in ow[:-1]:
                        _waitfix_counter[0] += 1
                        out.append({
                            "debug": insn.get("debug", 0),
                            "engine": insn["engine"],
                            "ins": [],
                            "name": f"{insn['name']}-wsplit{_waitfix_counter[0]}",
                            "opcode": "EventSemaphore",
                            "outs": [],
                            "sync_info": {"on_update": [], "on_wait": [w]},
                        })
                        nsplit += 1
                    si["on_wait"] = [ow[-1]]
                out.append(insn)
            blk["instructions"] = out
    return nsplit


def _install_waitfix():
    import concourse.bass2jax as bass2jax
    import concourse.bass_utils as bass_utils

    if getattr(bass2jax, "_waitfix_installed", False):
        return
    orig = bass_utils.compile_bir_kernel

    def patched(bir_json, tmpdir, neff_name="file.neff"):
        bir = json.loads(bir_json.decode() if isinstance(bir_json, bytes) else bir_json)
        _split_waits_in_bir(bir)
        return orig(json.dumps(bir).encode(), tmpdir, neff_name)

    bass2jax.compile_bir_kernel = patched
    bass_utils.compile_bir_kernel = patched
    bass2jax._waitfix_installed = True


def _softplus(x):
    return np.logaddexp(0.0, x.astype(np.float64)).astype(np.float32)


def _group_blocks(groups_sorted):
    """[(g, c0, c1)] contiguous column block (<=512 wide) per group g."""
    gs = np.asarray(groups_sorted)
    blocks = []
    for g in range(N_GROUPS):
        c0 = int(np.searchsorted(gs, g))
        c1 = int(np.searchsorted(gs, g + 1))
        while c0 < c1:
            ce = min(c0 + 512, c1)
            blocks.append((g, c0, ce))
            c0 = ce
    return blocks


def build_device_program(blocks):
    """One SPMD program (shared by all 8 cores). Per-core inputs:
      flatt [KCHUNKS, 128, BATCH]  — transposed feature slice (group-sorted)
      w     [128, KCHUNKS, 64]     — [W1|W2]*W_SCALE slice, partition-major
      rrt   [128, 8, 16, Z]        — (R1⊗R2) stationary: [ (dr1,r2), blk, g, z ]
      ffi   [128, 2, 16, Z] bf16   — stacked-identity * Ff[g, z, f1(j, p)]
    Outputs (partial sums over this core's features):
      zcat [64, BATCH] f32 — encoder output [W1|W2] partial
      t    [Z, BATCH] f32  — T partial (this core's f1 range)
    """
    nc = bass.Bass()
    flatt = nc.dram_tensor("flatt", [KCHUNKS, 128, BATCH], DT, kind="ExternalInput")
    w = nc.dram_tensor("w", [128, KCHUNKS, 64], DT, kind="ExternalInput")
    rrt = nc.dram_tensor("rrt", [128, 8, N_GROUPS, Z], DT, kind="ExternalInput")
    ffd = nc.dram_tensor("ffd", [Z, F1_PER_CORE, N_GROUPS, Z], BF16,
                         kind="ExternalInput")
    zcat_out = nc.dram_tensor("zcat", [64, BATCH], F32, kind="ExternalOutput")
    t_out = nc.dram_tensor("t", [Z, BATCH], F32, kind="ExternalOutput")

    DR = mybir.MatmulPerfMode.DoubleRow
    NPAIR = KCHUNKS // 2                     # 32

    with tile.TileContext(nc) as tc:
        with (
            tc.tile_pool(name="fpool", bufs=6) as fpool,
            tc.tile_pool(name="const", bufs=1) as const,
            tc.tile_pool(name="opool", bufs=1) as opool,
            tc.tile_pool(name="psum", bufs=1, space="PSUM") as psum,
        ):
            wt = const.tile([128, KCHUNKS, 64], DT, tag="w")
            rrt_sb = const.tile([128, 8, N_GROUPS, Z], DT, tag="rrt")
            ffd_sb = const.tile([Z, F1_PER_CORE, N_GROUPS, Z], BF16, tag="ffd")
            u_sb = [opool.tile([Z, BATCH], BF16, tag=f"u{j}", name=f"u_sb{j}")
                    for j in range(2)]

            zcat_ps = psum.tile([64, BATCH], F32, tag="zcat")
            u_ps = [psum.tile([Z, BATCH], F32, tag=f"u{j}", name=f"u_ps{j}")
                    for j in range(2)]
            t_ps = psum.tile([Z, BATCH], F32, tag="t")

            # --- pipeline: per pair, issue the ft DMA (with const slices
            # interleaved after the first few so PE can start early), then the
            # pair's matmuls. The 6-deep fpool lets DMA run ~6 pairs ahead.
            for p in range(NPAIR):
                k0 = 2 * p
                ft = fpool.tile([128, 2, BATCH], DT, tag="flat")
                nc.sync.dma_start(
                    out=ft, in_=flatt[k0:k0 + 2, :, :].rearrange("c p n -> p c n")
                )
                if p == 0:
                    nc.sync.dma_start(out=wt[:, 0:16, :], in_=w[:, 0:16, :])
                    nc.sync.dma_start(out=rrt_sb[:, 0:4, :, :], in_=rrt[:, 0:4, :, :])
                elif p == 1:
                    nc.sync.dma_start(out=rrt_sb[:, 4:8, :, :], in_=rrt[:, 4:8, :, :])
                elif p == 2:
                    nc.sync.dma_start(out=wt[:, 16:32, :], in_=w[:, 16:32, :])
                    nc.sync.dma_start(out=ffd_sb, in_=ffd[:, :, :, :])
                elif p == 3:
                    nc.sync.dma_start(out=wt[:, 32:64, :], in_=w[:, 32:64, :])
                for half in range(2):
                    nc.tensor.matmul(
                        zcat_ps[:, half * 512:(half + 1) * 512],
                        wt[:, k0:k0 + 2, :],
                        ft[:, :, half * 512:(half + 1) * 512],
                        start=(p == 0),
                        stop=(p == NPAIR - 1),
                        perf_mode=DR,
                    )
                f1loc = k0 // 8              # this core's f1 index (0..7)
                blk = k0 % 8                 # rr block pair (blk, blk+1)
                pp = f1loc % 2               # u_ps ping-pong slot
                for (g, c0, c1) in blocks:
                    nc.tensor.matmul(
                        u_ps[pp][:, c0:c1],
                        rrt_sb[:, blk:blk + 2, g, :],
                        ft[:, :, c0:c1],
                        start=(blk == 0),
                        stop=(blk == 6),
                        perf_mode=DR,
                    )
                if blk == 6:
                    # u for f1loc complete: narrow to bf16 (DVE/ACT alternate)
                    # then fold Ff via diag stationaries into t_ps. The
                    # ping-pong psum tile lets the next f1 accumulate while
                    # this one drains.
                    if pp == 0:
                        nc.vector.tensor_copy(u_sb[pp], u_ps[pp])
                    else:
                        nc.scalar.copy(u_sb[pp], u_ps[pp])
                    if p == NPAIR - 1:
                        # tail: drain zcat on the other engine in parallel
                        zc_sb = opool.tile([64, BATCH], F32, tag="zc")
                        nc.vector.tensor_copy(zc_sb, zcat_ps)
                        nc.sync.dma_start(out=zcat_out[:, :], in_=zc_sb)
                    for (g, c0, c1) in blocks:
                        nc.tensor.matmul(
                            t_ps[:, c0:c1],
                            ffd_sb[:, f1loc, g, :],
                            u_sb[pp][:, c0:c1],
                            start=(f1loc == 0),
                            stop=(f1loc == F1_PER_CORE - 1),
                        )

            t_sb = opool.tile([Z, BATCH], F32, tag="t")
            nc.scalar.copy(t_sb, t_ps)
            nc.sync.dma_start(out=t_out[:, :], in_=t_sb)
    return nc


def _prepare(inputs):
    features = np.asarray(inputs["features"], dtype=np.float32)
    labels = np.asarray(inputs["labels"]).astype(np.int64)
    groups = np.asarray(inputs["groups"]).astype(np.int64)
    weights = np.asarray(inputs["weights"], dtype=np.float32)
    noise = np.asarray(inputs["noise"], dtype=np.float32)
    group_embed = np.asarray(inputs["group_embed"], dtype=np.float32)
    W1 = np.asarray(inputs["W1"], dtype=np.float32)
    b1 = np.asarray(inputs["b1"], dtype=np.float32)
    W2 = np.asarray(inputs["W2"], dtype=np.float32)
    b2 = np.asarray(inputs["b2"], dtype=np.float32)
    freq_factors = np.asarray(inputs["freq_factors"], dtype=np.float32)
    roi_1_factors = np.asarray(inputs["roi_1_factors"], dtype=np.float32)
    roi_2_factors = np.asarray(inputs["roi_2_factors"], dtype=np.float32)
    lin_W = np.asarray(inputs["lin_W"], dtype=np.float32)
    lin_b = np.asarray(inputs["lin_b"], dtype=np.float32)
    logit_bias = np.asarray(inputs["logit_bias"], dtype=np.float32)

    b = features.shape[0]
    flat = features.reshape(b, -1)

    perm = np.argsort(groups, kind="stable")
    groups_sorted = groups[perm]
    blocks = _group_blocks(groups_sorted)

    sq = np.einsum("bi,bi->b", flat, flat, optimize=True)

    flat_q = flat[perm].astype(NPDT)
    flatT = flat_q.view(np.uint8).T.copy().view(NPDT)       # [N_FEAT, BATCH]

    W = (np.concatenate([W1[:N_FEAT], W2[:N_FEAT]], axis=1) * W_SCALE).astype(NPDT)

    Ff = _softplus(freq_factors)             # [16, 32z, 64f1]
    R1 = _softplus(roi_1_factors)            # [16, 32z, 32r1]
    R2 = _softplus(roi_2_factors)            # [16, 32z, 32r2]

    # rrt[p=(dr1,r2), blk, g, z] = R1[g,z,4*blk+dr1] * R2[g,z,r2]
    A = R1.reshape(N_GROUPS, Z, 8, 4)                       # [g, z, blk, dr1]
    rr = A[:, :, :, :, None] * R2[:, :, None, None, :]      # [g, z, blk, dr1, r2]
    rrt = np.ascontiguousarray(
        rr.transpose(3, 4, 2, 0, 1).reshape(128, 8, N_GROUPS, Z)
    ).astype(NPDT)

    w_dev = W.view(np.uint8).reshape(N_CORES, KCHUNKS, 128, 64)
    w_dev = w_dev.transpose(0, 2, 1, 3).copy().view(NPDT)

    # ffd[c][z', f1loc, g, z] = (z'==z) * Ff[g, z, 8c + f1loc]  (diag fold)
    eye = np.eye(Z, dtype=np.float32)                       # [z', z]
    ffd_all = np.zeros((N_CORES, Z, F1_PER_CORE, N_GROUPS, Z),
                       dtype=ml_dtypes.bfloat16)
    for c in range(N_CORES):
        for f1loc in range(F1_PER_CORE):
            # [z', g, z] = eye[z', z] * Ff[g, z, 8c + f1loc]
            ffd_all[c, :, f1loc] = eye[:, None, :] * Ff[None, :, :, 8 * c + f1loc]

    in_maps = []
    for c in range(N_CORES):
        in_maps.append({
            "flatt": np.ascontiguousarray(
                flatT[c * FEAT_PER_CORE:(c + 1) * FEAT_PER_CORE].view(np.uint8)
            ).reshape(KCHUNKS, 128, BATCH).view(NPDT),
            "w": w_dev[c],
            "rrt": rrt,
            "ffd": ffd_all[c],
        })

    host = dict(
        labels=labels, groups=groups, weights=weights, noise=noise,
        group_embed=group_embed, W1=W1, b1=b1, W2=W2, b2=b2,
        lin_W=lin_W, lin_b=lin_b, logit_bias=logit_bias,
        Ff=Ff, R1=R1, R2=R2, sq=sq, perm=perm, b=b,
        groups_sorted=groups_sorted,
    )
    return in_maps, blocks, host


def _finish(zcatT, ttT, host):
    b = host["b"]
    perm = host["perm"]
    inv = np.empty_like(perm)
    inv[perm] = np.arange(b)

    zcat = (zcatT / W_SCALE).T[inv]                        # [b, 64]
    T = ttT.T[inv]                                         # [b, Z]

    groups = host["groups"]
    ge = host["group_embed"][groups]
    z_mu = zcat[:, :Z] + host["b1"] + ge @ host["W1"][N_FEAT:]
    z_log_std = zcat[:, Z:] + host["b2"] + ge @ host["W2"][N_FEAT:]
    sigma = EPSILON + np.exp(z_log_std)
    kld = np.sum(-np.log(sigma) + 0.5 * (sigma * sigma + z_mu * z_mu - 1.0), axis=1)
    zs = z_mu + sigma * host["noise"]
    zs = zs @ host["lin_W"] + host["lin_b"]
    a = _softplus(zs)

    Ff, R1, R2 = host["Ff"], host["R1"], host["R2"]
    M = (np.einsum("gzf,gyf->gzy", Ff, Ff)
         * np.einsum("gzr,gyr->gzy", R1, R1)
         * np.einsum("gzs,gys->gzy", R2, R2))
    vol2 = np.einsum("bz,bzy,by->b", a, M[groups], a)
    fdotv = np.sum(a * T, axis=1)
    rec = REG_STRENGTH * (host["sq"] - 2.0 * fdotv + vol2) / N_FEAT

    logits = np.concatenate([zs[:, :N_CLASSES - 1], np.ones((b, 1), np.float32)],
                            axis=1) + host["logit_bias"]
    m = logits.max(axis=1, keepdims=True)
    lse = m[:, 0] + np.log(np.exp(logits - m).sum(axis=1))
    log_probs = logits[np.arange(b), host["labels"]] - lse

    freq_loss = np.var(Ff, axis=0, ddof=1).mean(axis=1).sum()
    roi_loss = (np.var(R1, axis=0, ddof=1) + np.var(R2, axis=0, ddof=1)).mean(axis=1).sum()

    loss = np.mean(rec - host["weights"] * log_probs + KL_FACTOR * kld) \
        + freq_loss + roi_loss
    return np.float32(loss)


def kernel(**inputs) -> np.ndarray:
    _install_waitfix()
    in_maps, blocks, host = _prepare(inputs)
    nc = build_device_program(blocks)
    r = run_bass_kernel_spmd(nc, in_maps, core_ids=list(range(N_CORES)))
    zcatT = np.zeros((64, BATCH), np.float32)
    ttT = np.zeros((Z, BATCH), np.float32)
    for c in range(N_CORES):
        zcatT += r.results[c]["zcat"]
        ttT += r.results[c]["t"]
    return _finish(zcatT, ttT, host)


# revision 26
# speedup vs baseline: 2.3674x; 1.0551x over previous
"""Trainium2 Bass kernel for nn_CpSae_44014824849572.

Computes the CP-SAE loss. The reference materializes a [1024, 64, 32, 32]
CP-reconstruction `volume` and diffs it against `features`. We instead use

  sum((flat - volume)^2) = sum(flat^2) - 2*sum(flat*volume) + sum(volume^2)

with  sum(flat*volume)[b] = sum_z a[b,z] * T[b,z],
      T[b,z]   = sum_feat flat[b,feat] * KRP[g_b][z,feat]
      KRP[g]   = softplus(freq)⊗softplus(roi1)⊗softplus(roi2)  (rank-1 rows)
      sum(volume^2)[b] = a_b^T M_{g_b} a_b,
      M_g = (Ff Ff^T) ∘ (R1 R1^T) ∘ (R2 R2^T)   (32x32 per group, tiny)

so the only heavy device work is two big contractions over the feature dim:
  zcat[b, 0:64] = flat[b] @ [W1 | W2]          (encoder, 8.6 GFLOP)
  T[b, z]       = flat[b] @ KRP[g_b].T         (4.3 GFLOP)

Instead of materializing KRP (4MB/core in HBM), we exploit its rank-1
structure: within a 128-feature chunk (f1 fixed, 4 r1 values x 32 r2) the
KRP stationary is (R1⊗R2)[g] scaled by Ff[g,z,f1]. The device contracts
against the f1-independent RR = R1⊗R2 stationary (0.5MB, shared by all
cores) accumulating per-f1 partials U[f1][z,b], then folds the Ff weights
with stacked-identity matmuls:  T[z,b] = sum_f1 Ff[g_b,z,f1] * U[f1][z,b].

Distribution: feature-dim sharded across 8 cores (8192 features = 8 f1
values each, all 1024 samples). Samples are group-sorted on the host so
each group's U-matmul sees a contiguous column block. All heavy matmuls are
fp8 with MatmulPerfMode.DoubleRow (two 128-feature chunks contracted per
instruction). Partial zcat/T are summed on host.
"""
import json

import numpy as np
import ml_dtypes

import concourse.bass as bass
import concourse.mybir as mybir
import concourse.tile as tile
from concourse.bass_utils import run_bass_kernel_spmd

N_CORES = 8
BATCH = 1024
N_FREQS = 64
N_ROIS = 32
Z = 32
N_GROUPS = 16
N_CLASSES = 4
N_FEAT = N_FREQS * N_ROIS * N_ROIS          # 65536
FEAT_PER_CORE = N_FEAT // N_CORES           # 8192
KCHUNKS = FEAT_PER_CORE // 128              # 64
F1_PER_CORE = FEAT_PER_CORE // (N_ROIS * N_ROIS)  # 8
EPSILON = 1e-06
REG_STRENGTH = 1.0
KL_FACTOR = 1.0

F32 = mybir.dt.float32
BF16 = mybir.dt.bfloat16
DT = mybir.dt.float8e4
NPDT = ml_dtypes.float8_e4m3
W_SCALE = 4096.0

_waitfix_counter = [0]


def _split_waits_in_bir(bir: dict) -> int:
    """This container's walrus accepts only ONE sync wait per instruction;
    Tile emits several. Hoist all-but-one wait onto EventSemaphore
    instructions inserted just before, on the same engine."""
    nsplit = 0
    for fn in bir.get("functions", []):
        for blk in fn.get("blocks", []):
            out = []
            for insn in blk.get("instructions", []):
                si = insn.get("sync_info") or {}
                ow = si.get("on_wait") or []
                if len(ow) > 1:
                    for w in ow[:-1]:
                        _waitfix_counter[0] += 1
                        out.append({
                            "debug": insn.get("debug", 0),
                            "engine": insn["engine"],
                            "ins": [],
                            "name": f"{insn['name']}-wsplit{_waitfix_counter[0]}",
                            "opcode": "EventSemaphore",
                            "outs": [],
                            "sync_info": {"on_update": [], "on_wait": [w]},
                        })
                        nsplit += 1
                    si["on_wait"] = [ow[-1]]
                out.append(insn)
            blk["instructions"] = out
    return nsplit


def _install_waitfix():
    import concourse.bass2jax as bass2jax
    import concourse.bass_utils as bass_utils

    if getattr(bass2jax, "_waitfix_installed", False):
        return
    orig = bass_utils.compile_bir_kernel

    def patched(bir_json, tmpdir, neff_name="file.neff"):
        bir = json.loads(bir_json.decode() if isinstance(bir_json, bytes) else bir_json)
        _split_waits_in_bir(bir)
        return orig(json.dumps(bir).encode(), tmpdir, neff_name)

    bass2jax.compile_bir_kernel = patched
    bass_utils.compile_bir_kernel = patched
    bass2jax._waitfix_installed = True


def _softplus(x):
    return np.logaddexp(0.0, x.astype(np.float64)).astype(np.float32)


def _group_blocks(groups_sorted):
    """[(g, c0, c1)] contiguous column block (<=512 wide) per group g."""
    gs = np.asarray(groups_sorted)
    blocks = []
    for g in range(N_GROUPS):
        c0 = int(np.searchsorted(gs, g))
        c1 = int(np.searchsorted(gs, g + 1))
        while c0 < c1:
            ce = min(c0 + 512, c1)
            blocks.append((g, c0, ce))
            c0 = ce
    return blocks


def build_device_program(blocks):
    """One SPMD program (shared by all 8 cores). Per-core inputs:
      flatt [KCHUNKS, 128, BATCH]  — transposed feature slice (group-sorted)
      w     [128, KCHUNKS, 64]     — [W1|W2]*W_SCALE slice, partition-major
      rrt   [128, 8, 16, Z]        — (R1⊗R2) stationary: [ (dr1,r2), blk, g, z ]
      ffi   [128, 2, 16, Z] bf16   — stacked-identity * Ff[g, z, f1(j, p)]
    Outputs (partial sums over this core's features):
      zcat [64, BATCH] f32 — encoder output [W1|W2] partial
      t    [Z, BATCH] f32  — T partial (this core's f1 range)
    """
    nc = bass.Bass()
    flatt = nc.dram_tensor("flatt", [KCHUNKS, 128, BATCH], DT, kind="ExternalInput")
    w = nc.dram_tensor("w", [128, KCHUNKS, 64], DT, kind="ExternalInput")
    rrt = nc.dram_tensor("rrt", [128, 8, N_GROUPS, Z], DT, kind="ExternalInput")
    ffd = nc.dram_tensor("ffd", [Z, F1_PER_CORE, N_GROUPS, Z], BF16,
                         kind="ExternalInput")
    zcat_out = nc.dram_tensor("zcat", [64, BATCH], BF16, kind="ExternalOutput")
    t_out = nc.dram_tensor("t", [Z, BATCH], BF16, kind="ExternalOutput")
    u7_out = nc.dram_tensor("u7", [Z, BATCH], BF16, kind="ExternalOutput")

    DR = mybir.MatmulPerfMode.DoubleRow
    NPAIR = KCHUNKS // 2                     # 32

    with tile.TileContext(nc) as tc:
        with (
            tc.tile_pool(name="fpool", bufs=8) as fpool,
            tc.tile_pool(name="const", bufs=1) as const,
            tc.tile_pool(name="opool", bufs=1) as opool,
            tc.tile_pool(name="psum", bufs=1, space="PSUM") as psum,
        ):
            wt = const.tile([128, KCHUNKS, 64], DT, tag="w")
            rrt_sb = const.tile([128, 8, N_GROUPS, Z], DT, tag="rrt")
            ffd_sb = const.tile([Z, F1_PER_CORE, N_GROUPS, Z], BF16, tag="ffd")
            u_sb = [opool.tile([Z, BATCH], BF16, tag=f"u{j}", name=f"u_sb{j}")
                    for j in range(2)]

            zcat_ps = psum.tile([64, BATCH], F32, tag="zcat")
            u_ps = [psum.tile([Z, BATCH], F32, tag=f"u{j}", name=f"u_ps{j}")
                    for j in range(2)]
            t_ps = psum.tile([Z, BATCH], F32, tag="t")

            # --- pipeline: per pair, issue the ft DMA (with const slices
            # interleaved after the first few so PE can start early), then the
            # pair's matmuls. The 8-deep fpool lets DMA run ~8 pairs ahead.
            # Folds are deferred 2 pairs after their u-copy so PE never stalls.
            pending_fold = []
            for p in range(NPAIR):
                k0 = 2 * p
                ft = fpool.tile([128, 2, BATCH], DT, tag="flat")
                nc.sync.dma_start(
                    out=ft, in_=flatt[k0:k0 + 2, :, :].rearrange("c p n -> p c n")
                )
                if p == 0:
                    nc.sync.dma_start(out=wt[:, 0:16, :], in_=w[:, 0:16, :])
                    nc.sync.dma_start(out=rrt_sb[:, 0:4, :, :], in_=rrt[:, 0:4, :, :])
                elif p == 1:
                    nc.sync.dma_start(out=rrt_sb[:, 4:8, :, :], in_=rrt[:, 4:8, :, :])
                elif p == 2:
                    nc.sync.dma_start(out=wt[:, 16:32, :], in_=w[:, 16:32, :])
                    nc.sync.dma_start(out=ffd_sb, in_=ffd[:, :, :, :])
                elif p == 3:
                    nc.sync.dma_start(out=wt[:, 32:64, :], in_=w[:, 32:64, :])
                for half in range(2):
                    nc.tensor.matmul(
                        zcat_ps[:, half * 512:(half + 1) * 512],
                        wt[:, k0:k0 + 2, :],
                        ft[:, :, half * 512:(half + 1) * 512],
                        start=(p == 0),
                        stop=(p == NPAIR - 1),
                        perf_mode=DR,
                    )
                f1loc = k0 // 8              # this core's f1 index (0..7)
                blk = k0 % 8                 # rr block pair (blk, blk+1)
                pp = f1loc % 2               # u_ps ping-pong slot
                for (g, c0, c1) in blocks:
                    nc.tensor.matmul(
                        u_ps[pp][:, c0:c1],
                        rrt_sb[:, blk:blk + 2, g, :],
                        ft[:, :, c0:c1],
                        start=(blk == 0),
                        stop=(blk == 6),
                        perf_mode=DR,
                    )
                if pending_fold and pending_fold[0][1] == p:
                    f1d, _, ppd = pending_fold.pop(0)
                    for (g, c0, c1) in blocks:
                        nc.tensor.matmul(
                            t_ps[:, c0:c1],
                            ffd_sb[:, f1d, g, :],
                            u_sb[ppd][:, c0:c1],
                            start=(f1d == 0),
                            stop=(f1d == F1_PER_CORE - 2),
                        )
                    if f1d == F1_PER_CORE - 2:
                        # t (f1 0..6) complete mid-stream: narrow it now; its
                        # DMA is issued in the tail so the in-order sync queue
                        # never blocks the remaining ft transfers behind it.
                        t_sb = opool.tile([Z, BATCH], BF16, tag="t")
                        nc.scalar.copy(t_sb, t_ps)
                if blk == 6 and f1loc < F1_PER_CORE - 1:
                    # u for f1loc complete: narrow to bf16 (DVE/ACT alternate);
                    # the diag-Ff fold into t_ps is deferred 2 pairs.
                    if pp == 0:
                        nc.vector.tensor_copy(u_sb[pp], u_ps[pp])
                    else:
                        nc.scalar.copy(u_sb[pp], u_ps[pp])
                    pending_fold.append((f1loc, p + 2, pp))

            # --- tail: the last f1's u ships raw (host folds Ff for it); zcat
            # and u7 copies are column-split across DVE/ACT.
            nc.sync.dma_start(out=t_out[:, :], in_=t_sb)
            zc_sb = opool.tile([64, BATCH], BF16, tag="zc")
            u7_sb = opool.tile([Z, BATCH], BF16, tag="u7")
            H = BATCH // 2
            nc.vector.tensor_copy(zc_sb[:, 0:H], zcat_ps[:, 0:H])
            nc.scalar.copy(zc_sb[:, H:], zcat_ps[:, H:])
            nc.sync.dma_start(out=zcat_out[:, :], in_=zc_sb)
            nc.vector.tensor_copy(u7_sb[:, 0:H], u_ps[1][:, 0:H])
            nc.scalar.copy(u7_sb[:, H:], u_ps[1][:, H:])
            nc.sync.dma_start(out=u7_out[:, :], in_=u7_sb)
    return nc


def _prepare(inputs):
    features = np.asarray(inputs["features"], dtype=np.float32)
    labels = np.asarray(inputs["labels"]).astype(np.int64)
    groups = np.asarray(inputs["groups"]).astype(np.int64)
    weights = np.asarray(inputs["weights"], dtype=np.float32)
    noise = np.asarray(inputs["noise"], dtype=np.float32)
    group_embed = np.asarray(inputs["group_embed"], dtype=np.float32)
    W1 = np.asarray(inputs["W1"], dtype=np.float32)
    b1 = np.asarray(inputs["b1"], dtype=np.float32)
    W2 = np.asarray(inputs["W2"], dtype=np.float32)
    b2 = np.asarray(inputs["b2"], dtype=np.float32)
    freq_factors = np.asarray(inputs["freq_factors"], dtype=np.float32)
    roi_1_factors = np.asarray(inputs["roi_1_factors"], dtype=np.float32)
    roi_2_factors = np.asarray(inputs["roi_2_factors"], dtype=np.float32)
    lin_W = np.asarray(inputs["lin_W"], dtype=np.float32)
    lin_b = np.asarray(inputs["lin_b"], dtype=np.float32)
    logit_bias = np.asarray(inputs["logit_bias"], dtype=np.float32)

    b = features.shape[0]
    flat = features.reshape(b, -1)

    perm = np.argsort(groups, kind="stable")
    groups_sorted = groups[perm]
    blocks = _group_blocks(groups_sorted)

    sq = np.einsum("bi,bi->b", flat, flat, optimize=True)

    flat_q = flat[perm].astype(NPDT)
    flatT = flat_q.view(np.uint8).T.copy().view(NPDT)       # [N_FEAT, BATCH]

    W = (np.concatenate([W1[:N_FEAT], W2[:N_FEAT]], axis=1) * W_SCALE).astype(NPDT)

    Ff = _softplus(freq_factors)             # [16, 32z, 64f1]
    R1 = _softplus(roi_1_factors)            # [16, 32z, 32r1]
    R2 = _softplus(roi_2_factors)            # [16, 32z, 32r2]

    # rrt[p=(dr1,r2), blk, g, z] = R1[g,z,4*blk+dr1] * R2[g,z,r2]
    A = R1.reshape(N_GROUPS, Z, 8, 4)                       # [g, z, blk, dr1]
    rr = A[:, :, :, :, None] * R2[:, :, None, None, :]      # [g, z, blk, dr1, r2]
    rrt = np.ascontiguousarray(
        rr.transpose(3, 4, 2, 0, 1).reshape(128, 8, N_GROUPS, Z)
    ).astype(NPDT)

    w_dev = W.view(np.uint8).reshape(N_CORES, KCHUNKS, 128, 64)
    w_dev = w_dev.transpose(0, 2, 1, 3).copy().view(NPDT)

    # ffd[c][z', f1loc, g, z] = (z'==z) * Ff[g, z, 8c + f1loc]  (diag fold)
    eye = np.eye(Z, dtype=np.float32)                       # [z', z]
    ffd_all = np.zeros((N_CORES, Z, F1_PER_CORE, N_GROUPS, Z),
                       dtype=ml_dtypes.bfloat16)
    for c in range(N_CORES):
        for f1loc in range(F1_PER_CORE):
            # [z', g, z] = eye[z', z] * Ff[g, z, 8c + f1loc]
            ffd_all[c, :, f1loc] = eye[:, None, :] * Ff[None, :, :, 8 * c + f1loc]

    in_maps = []
    for c in range(N_CORES):
        in_maps.append({
            "flatt": np.ascontiguousarray(
                flatT[c * FEAT_PER_CORE:(c + 1) * FEAT_PER_CORE].view(np.uint8)
            ).reshape(KCHUNKS, 128, BATCH).view(NPDT),
            "w": w_dev[c],
            "rrt": rrt,
            "ffd": ffd_all[c],
        })

    host = dict(
        labels=labels, groups=groups, weights=weights, noise=noise,
        group_embed=group_embed, W1=W1, b1=b1, W2=W2, b2=b2,
        lin_W=lin_W, lin_b=lin_b, logit_bias=logit_bias,
        Ff=Ff, R1=R1, R2=R2, sq=sq, perm=perm, b=b,
        groups_sorted=groups_sorted,
    )
    return in_maps, blocks, host


def _finish(zcatT, ttT, host):
    b = host["b"]
    perm = host["perm"]
    inv = np.empty_like(perm)
    inv[perm] = np.arange(b)

    zcat = (zcatT / W_SCALE).T[inv]                        # [b, 64]
    T = ttT.T[inv]                                         # [b, Z]

    groups = host["groups"]
    ge = host["group_embed"][groups]
    z_mu = zcat[:, :Z] + host["b1"] + ge @ host["W1"][N_FEAT:]
    z_log_std = zcat[:, Z:] + host["b2"] + ge @ host["W2"][N_FEAT:]
    sigma = EPSILON + np.exp(z_log_std)
    kld = np.sum(-np.log(sigma) + 0.5 * (sigma * sigma + z_mu * z_mu - 1.0), axis=1)
    zs = z_mu + sigma * host["noise"]
    zs = zs @ host["lin_W"] + host["lin_b"]
    a = _softplus(zs)

    Ff, R1, R2 = host["Ff"], host["R1"], host["R2"]
    M = (np.einsum("gzf,gyf->gzy", Ff, Ff)
         * np.einsum("gzr,gyr->gzy", R1, R1)
         * np.einsum("gzs,gys->gzy", R2, R2))
    vol2 = np.einsum("bz,bzy,by->b", a, M[groups], a)
    fdotv = np.sum(a * T, axis=1)
    rec = REG_STRENGTH * (host["sq"] - 2.0 * fdotv + vol2) / N_FEAT

    logits = np.concatenate([zs[:, :N_CLASSES - 1], np.ones((b, 1), np.float32)],
                            axis=1) + host["logit_bias"]
    m = logits.max(axis=1, keepdims=True)
    lse = m[:, 0] + np.log(np.exp(logits - m).sum(axis=1))
    log_probs = logits[np.arange(b), host["labels"]] - lse

    freq_loss = np.var(Ff, axis=0, ddof=1).mean(axis=1).sum()
    roi_loss = (np.var(R1, axis=0, ddof=1) + np.var(R2, axis=0, ddof=1)).mean(axis=1).sum()

    loss = np.mean(rec - host["weights"] * log_probs + KL_FACTOR * kld) \
        + freq_loss + roi_loss
    return np.float32(loss)


def kernel(**inputs) -> np.ndarray:
    _install_waitfix()
    in_maps, blocks, host = _prepare(inputs)
    nc = build_device_program(blocks)
    r = run_bass_kernel_spmd(nc, in_maps, core_ids=list(range(N_CORES)))
    zcatT = np.zeros((64, BATCH), np.float32)
    ttT = np.zeros((Z, BATCH), np.float32)
    # per-column Ff factor for each core's last f1 (u7 is shipped unfolded)
    f7 = host["Ff"][host["groups_sorted"]]            # [col, z, 64]
    for c in range(N_CORES):
        zcatT += np.asarray(r.results[c]["zcat"], dtype=np.float32)
        ttT += np.asarray(r.results[c]["t"], dtype=np.float32)
        ttT += f7[:, :, 8 * c + 7].T * np.asarray(r.results[c]["u7"],
                                                  dtype=np.float32)
    return _finish(zcatT, ttT, host)
